# revision 1
# baseline (speedup 1.0000x reference)
"""Trainium2 Bass kernel for nn_BilinearFeedForward.

Math (per batch element b, reference semantics):
    q_r = x @ Wqr ; q_i = x @ Wqi ; query = relu(q_r) * relu(q_i)
    k = x @ Wk ; v = x @ Wv
    k /= (||k||_2 over n + eps) ; v /= (||v||_2 over n + eps)   (per column)
    kv = relu(k^T v)            [D, D]
    out = query @ kv            [N, D]

Key algebraic restructuring: with G = x^T x (symmetric, [D, D])
    k^T v       = Wk^T G Wv
    ||k_e||^2   = diag(Wk^T G Wk)_e ,  ||v_e||^2 = diag(Wv^T G Wv)_e
so k and v ([N, D] each) are never materialized; the sequence-length
reduction happens once inside G (upper-triangle blocks only, G symmetric).
rnk = 1/(nk+eps) folds into the relu-copy of KV as a per-partition ACT
scale (relu commutes with positive scaling); rnv = 1/(nv+eps) is a pure
column scaling of the final output.  query is produced transposed
(query^T, [D, N]) so the final einsum consumes it as the matmul stationary
operand directly.

All big matmuls run as float32r (full PE rate; fp32 is 4x slower).  The
BIR verifier requires f32r matmul inputs to be produced as f32r, so the
feeding DRAM tensors / SBUF tiles are declared float32r (same 4-byte bits).

Sharding: data-parallel over batch — 8 batch elements -> 8 NeuronCores,
weights replicated.  No collectives.
"""

import os

import numpy as np

import concourse.bass as bass
import concourse.mybir as mybir
import concourse.tile as tile
from concourse.bass_utils import run_bass_kernel_spmd
from concourse.masks import make_identity

F32 = mybir.dt.float32
F32R = mybir.dt.float32r
RELU = mybir.ActivationFunctionType.Relu
SQRT = mybir.ActivationFunctionType.Sqrt

B, N, D = 8, 4096, 1024
P = 128
DC = D // P          # 8 feature chunks
SLAB = 512           # token slab
NSLAB = N // SLAB    # 8
EPS = 1e-05

# G = x^T x upper-triangle piece tables: (rowblock i, colstart, width, bank, bankoff)
# Row-block i covers G[128i:128(i+1), 128i:1024]; pieces are matmul moving slices
# packed into [128, 512] f32 PSUM bank tiles.
G_PIECES_A = [  # row blocks 0..3 -> 7 banks (phase A)
    (0, 0, 512, 0, 0), (0, 512, 512, 1, 0),
    (1, 128, 512, 2, 0), (1, 640, 384, 3, 0),
    (3, 384, 128, 3, 384), (3, 512, 512, 4, 0),
    (2, 256, 512, 5, 0), (2, 768, 256, 6, 0),
]
N_BANKS_A = 7
G_PIECES_B = [  # row blocks 4..7 -> 3 banks (phase B)
    (4, 512, 512, 0, 0),
    (5, 640, 384, 1, 0), (7, 896, 128, 1, 384),
    (6, 768, 256, 2, 0),
]
N_BANKS_B = 3

last_exec_time_ns = None
last_results = None


def _split_multi_waits(nc, max_waits=1):
    """This container's walrus accepts at most ONE sync-wait per instruction
    ("Too many sync wait commands" otherwise).  Tile attaches several, so
    move the extras onto injected same-engine NoOps placed just before each
    offending instruction — engine streams dispatch in order, so a leading
    nop that blocks on the extra conditions is semantically identical."""
    ctr = 0
    for func in nc.m.functions:
        for bb in func.blocks:
            out = []
            changed = False
            for inst in bb.instructions:
                si = inst.sync_info
                waits = list(si.on_wait) if si and si.on_wait else []
                if len(waits) > max_waits:
                    for w in waits[:-max_waits]:
                        ctr += 1
                        nop = mybir.InstNoOp(
                            name=f"I-waitsplit-{ctr}",
                            engine=inst.engine,
                            sync_info=mybir.SyncInfo(on_wait=[w], on_update=[]),
                        )
                        nc.register_instruction(nop)
                        out.append(nop)
                    inst.sync_info = mybir.SyncInfo(
                        on_wait=waits[-max_waits:],
                        on_update=list(si.on_update) if si.on_update else [],
                    )
                    changed = True
                out.append(inst)
            if changed:
                bb.instructions = out
    return ctr


def _copy_r(nc, idx, out_r, in_ps):
    """PSUM -> f32r SBUF copy, alternating DVE / ACT.
    DVE path reads the source as f32r (f32r->f32r copy); ACT path reads it
    as f32 and casts on writeback (both verified on HW)."""
    if idx % 2 == 0:
        nc.vector.tensor_copy(out_r, in_ps.bitcast(F32R))
    else:
        nc.scalar.copy(out_r, in_ps.bitcast(F32))


def _load_weight(nc, pool, w_dram, name):
    """[D, D] f32r weight -> SBUF [128, DC, D] (partition = row-within-chunk),
    one DMA per row chunk so consumers can start before the full load."""
    t = pool.tile([P, DC, D], F32R, tag=name)
    wr = w_dram.rearrange("(c p) e -> p c e", p=P)
    for c in range(DC):
        nc.sync.dma_start(t[:, c, :], wr[:, c, :])
    return t


_DEBUG_DUMP = bool(os.environ.get("KERNEL_DEBUG_DUMP"))


def _build_program():
    # all data DMAs here are HWDGE (nc.sync); shrink the SWDGE descriptor-ring
    # SBUF carveout from its 16 KiB/partition default to reclaim SBUF
    nc = bass.Bass(dynamic_dma_scratch_size=2048)

    x_d = nc.dram_tensor("x", [N, D], F32R, kind="ExternalInput")
    wqr_d = nc.dram_tensor("w_query_real", [D, D], F32R, kind="ExternalInput")
    wqi_d = nc.dram_tensor("w_query_imag", [D, D], F32R, kind="ExternalInput")
    wk_d = nc.dram_tensor("w_key", [D, D], F32R, kind="ExternalInput")
    wv_d = nc.dram_tensor("w_value", [D, D], F32R, kind="ExternalInput")
    y_d = nc.dram_tensor("y", [N, D], F32, kind="ExternalOutput")

    x_r = x_d.rearrange("(s t p) d -> s p t d", p=P, t=SLAB // P)  # [8, 128, 4, 1024]

    with tile.TileContext(nc) as tc:
        with (
            tc.tile_pool(name="consts", bufs=1) as consts,
            tc.tile_pool(name="gsb", bufs=1) as gsb_pool,
            tc.tile_pool(name="asb", bufs=1) as a_pool,
            tc.tile_pool(name="vecs", bufs=1) as vecs_pool,
            tc.tile_pool(name="dram", bufs=1, space="DRAM") as dram_pool,
        ):
            ident_f = consts.tile([P, P], F32)
            make_identity(nc, ident_f)
            ident = consts.tile([P, P], F32R)
            nc.sync.dma_start(ident[:], ident_f[:].bitcast(F32R))
            ones = consts.tile([P, 1], F32)
            nc.vector.memset(ones, 1.0)

            g_sb = gsb_pool.tile([P, DC, D], F32R)  # full G, filled in pieces
            rnv_b = vecs_pool.tile([P, D], F32)     # 1/(nv+eps) bcast (phase D)
            qT_dram = dram_pool.tile([D, N], F32R)  # query^T spill

            with tc.tile_pool(name="wq", bufs=1) as wq_pool:
                # the two query-projection weights (used in phase B)
                wqr_sb = _load_weight(nc, wq_pool, wqr_d, "wqr")
                wqi_sb = _load_weight(nc, wq_pool, wqi_d, "wqi")

                # ---------------- Phase A: G row-blocks 0..3 ----------------
                with (
                    tc.tile_pool(name="xa", bufs=3) as xa_pool,
                    tc.tile_pool(name="psA", bufs=1, space="PSUM") as psA,
                ):
                    ga = [psA.tile([P, 512], F32, tag=f"ga{k}", name=f"ga{k}") for k in range(N_BANKS_A)]
                    for s in range(NSLAB):
                        xa = xa_pool.tile([P, SLAB // P, D], F32R, tag="xa")
                        nc.sync.dma_start(xa[:], x_r[s])
                        for t in range(SLAB // P):
                            for (i, cs, w, bk, off) in G_PIECES_A:
                                nc.tensor.matmul(
                                    ga[bk][:, off : off + w],
                                    xa[:, t, i * P : (i + 1) * P],
                                    xa[:, t, cs : cs + w],
                                    start=(s == 0 and t == 0),
                                    stop=(s == NSLAB - 1 and t == SLAB // P - 1),
                                )
                    # flush G rows 0..3 to SBUF
                    for n_, (i, cs, w, bk, off) in enumerate(G_PIECES_A):
                        _copy_r(nc, n_, g_sb[:, i, cs : cs + w], ga[bk][:, off : off + w])

                # ------- Phase B: transpose + query^T + G rows 4..7 ---------
                with (
                    tc.tile_pool(name="xb", bufs=2) as xb_pool,
                    tc.tile_pool(name="xt", bufs=2) as xt_pool,
                    tc.tile_pool(name="qre", bufs=2) as qre_pool,
                    tc.tile_pool(name="qto", bufs=3) as qto_pool,
                    tc.tile_pool(name="psB", bufs=1, space="PSUM") as psB,
                    tc.tile_pool(name="pt", bufs=2, space="PSUM") as pt_pool,
                    tc.tile_pool(name="pq", bufs=3, space="PSUM") as pq_pool,
                ):
                    gb = [psB.tile([P, 512], F32, tag=f"gb{k}", name=f"gb{k}") for k in range(N_BANKS_B)]
                    for s in range(NSLAB):
                        n0 = s * SLAB
                        xb = xb_pool.tile([P, SLAB // P, D], F32R, tag="xb")
                        nc.sync.dma_start(xb[:], x_r[s])

                        # transpose x slab -> x^T slab [128(d), DC, SLAB]
                        xt = xt_pool.tile([P, DC, SLAB], F32R, tag="xt")
                        for dc in range(DC):
                            ptile = pt_pool.tile([P, SLAB], F32R, tag="pt", name="pt")
                            for t in range(SLAB // P):
                                nc.tensor.transpose(
                                    ptile[:, t * P : (t + 1) * P],
                                    xb[:, t, dc * P : (dc + 1) * P],
                                    ident,
                                )
                            _copy_r(nc, dc, xt[:, dc, :], ptile[:].bitcast(F32))

                        # G row-blocks 4..7 accumulation
                        for t in range(SLAB // P):
                            for (i, cs, w, bk, off) in G_PIECES_B:
                                nc.tensor.matmul(
                                    gb[bk][:, off : off + w],
                                    xb[:, t, i * P : (i + 1) * P],
                                    xb[:, t, cs : cs + w],
                                    start=(s == 0 and t == 0),
                                    stop=(s == NSLAB - 1 and t == SLAB // P - 1),
                                )

                        # query^T = relu(Wqr^T x^T) * relu(Wqi^T x^T)
                        for ec in range(DC):
                            pr = pq_pool.tile([P, SLAB], F32, tag="pq", name="pr")
                            for dc in range(DC):
                                nc.tensor.matmul(
                                    pr[:],
                                    wqr_sb[:, dc, ec * P : (ec + 1) * P],
                                    xt[:, dc, :],
                                    start=(dc == 0),
                                    stop=(dc == DC - 1),
                                )
                            pi = pq_pool.tile([P, SLAB], F32, tag="pq", name="pi")
                            for dc in range(DC):
                                nc.tensor.matmul(
                                    pi[:],
                                    wqi_sb[:, dc, ec * P : (ec + 1) * P],
                                    xt[:, dc, :],
                                    start=(dc == 0),
                                    stop=(dc == DC - 1),
                                )
                            rr = qre_pool.tile([P, SLAB], F32, tag="rr")
                            nc.scalar.activation(rr[:], pr[:], RELU)
                            ri = qre_pool.tile([P, SLAB], F32, tag="ri")
                            nc.scalar.activation(ri[:], pi[:], RELU)
                            qt = qto_pool.tile([P, SLAB], F32, tag="qt")
                            nc.vector.tensor_mul(qt[:], rr[:], ri[:])
                            nc.sync.dma_start(
                                qT_dram[ec * P : (ec + 1) * P, n0 : n0 + SLAB],
                                qt[:].bitcast(F32R),
                            )
                    # flush G rows 4..7
                    for n_, (i, cs, w, bk, off) in enumerate(G_PIECES_B):
                        _copy_r(nc, n_, g_sb[:, i, cs : cs + w], gb[bk][:, off : off + w])

            # mirror the strictly-upper blocks of G into the lower triangle
            with tc.tile_pool(name="ptC", bufs=2, space="PSUM") as ptC_pool:
                for i in range(DC):
                    for j in range(i + 1, DC):
                        ptile = ptC_pool.tile([P, P], F32R, tag="ptc", name="ptc")
                        nc.tensor.transpose(
                            ptile[:], g_sb[:, i, j * P : (j + 1) * P], ident
                        )
                        _copy_r(nc, i + j, g_sb[:, j, i * P : (i + 1) * P],
                                ptile[:].bitcast(F32))

            # ---------------- Phase C: KV attention matrix A ----------------
            with (
                tc.tile_pool(name="wkv", bufs=1) as wkv_pool,
                tc.tile_pool(name="mv", bufs=1) as mv_pool,
                tc.tile_pool(name="ctmp", bufs=2) as ctmp_pool,
                tc.tile_pool(name="cvec", bufs=1) as cvec_pool,
                tc.tile_pool(name="psC", bufs=3, space="PSUM") as psC_pool,
                tc.tile_pool(name="pnrm", bufs=1, space="PSUM") as pnrm_pool,
            ):
                a_sb = a_pool.tile([P, DC, D], F32R)  # relu'd row-scaled KV
                wk_sb = _load_weight(nc, wkv_pool, wk_d, "wk")
                wv_sb = _load_weight(nc, wkv_pool, wv_d, "wv")
                mv_sb = mv_pool.tile([P, DC, D], F32R)

                # Mv = G Wv   [D, D]
                for mc in range(DC):
                    for eh in range(2):
                        pm = psC_pool.tile([P, 512], F32, tag="psc", name="pm")
                        for dc in range(DC):
                            nc.tensor.matmul(
                                pm[:],
                                g_sb[:, dc, mc * P : (mc + 1) * P],
                                wv_sb[:, dc, eh * 512 : (eh + 1) * 512],
                                start=(dc == 0),
                                stop=(dc == DC - 1),
                            )
                        _copy_r(nc, eh, mv_sb[:, mc, eh * 512 : (eh + 1) * 512], pm[:])

                # nv^2 = colsum(Wv * Mv) ; rnv = 1/(sqrt(nv^2)+eps)
                pnv = [pnrm_pool.tile([1, 512], F32, tag=f"pnv{h}", name=f"pnv{h}") for h in range(2)]
                for mc in range(DC):
                    tmpv = ctmp_pool.tile([P, D], F32, tag="tmpv")
                    nc.vector.tensor_mul(
                        tmpv[:],
                        wv_sb[:, mc, :].bitcast(F32),
                        mv_sb[:, mc, :].bitcast(F32),
                    )
                    for eh in range(2):
                        nc.tensor.matmul(
                            pnv[eh][:],
                            ones[:, 0:1],
                            tmpv[:, eh * 512 : (eh + 1) * 512],
                            start=(mc == 0),
                            stop=(mc == DC - 1),
                        )
                rnv_row = cvec_pool.tile([1, D], F32, tag="rnv_row")
                for eh in range(2):
                    nc.scalar.activation(
                        rnv_row[:, eh * 512 : (eh + 1) * 512], pnv[eh][:], SQRT
                    )
                nc.vector.tensor_scalar_add(rnv_row[:], rnv_row[:], EPS)
                nc.vector.reciprocal(rnv_row[:], rnv_row[:])
                # broadcast [1, D] across partitions via DRAM bounce
                nv_dram = dram_pool.tile([1, D], F32)
                nc.sync.dma_start(nv_dram[:], rnv_row[:])
                nc.sync.dma_start(rnv_b[:], nv_dram[0:1, :].to_broadcast((P, D)))

                # nk^2 via Mk = G Wk (not materialized) ; rnk
                pnk = [pnrm_pool.tile([1, 512], F32, tag=f"pnk{h}", name=f"pnk{h}") for h in range(2)]
                for mc in range(DC):
                    tmpk = ctmp_pool.tile([P, D], F32, tag="tmpk")
                    for eh in range(2):
                        pk = psC_pool.tile([P, 512], F32, tag="psc", name="pk")
                        for dc in range(DC):
                            nc.tensor.matmul(
                                pk[:],
                                g_sb[:, dc, mc * P : (mc + 1) * P],
                                wk_sb[:, dc, eh * 512 : (eh + 1) * 512],
                                start=(dc == 0),
                                stop=(dc == DC - 1),
                            )
                        nc.vector.tensor_mul(
                            tmpk[:, eh * 512 : (eh + 1) * 512],
                            wk_sb[:, mc, eh * 512 : (eh + 1) * 512].bitcast(F32),
                            pk[:],
                        )
                    for eh in range(2):
                        nc.tensor.matmul(
                            pnk[eh][:],
                            ones[:, 0:1],
                            tmpk[:, eh * 512 : (eh + 1) * 512],
                            start=(mc == 0),
                            stop=(mc == DC - 1),
                        )
                rnk_row = cvec_pool.tile([1, D], F32, tag="rnk_row")
                for eh in range(2):
                    nc.scalar.activation(
                        rnk_row[:, eh * 512 : (eh + 1) * 512], pnk[eh][:], SQRT
                    )
                nc.vector.tensor_scalar_add(rnk_row[:], rnk_row[:], EPS)
                nc.vector.reciprocal(rnk_row[:], rnk_row[:])
                nk_dram = dram_pool.tile([1, D], F32)
                nc.sync.dma_start(nk_dram[:], rnk_row[:])
                # rnk in per-partition layout [128, DC]: partition p <-> e_k = ec*128+p
                rnk_part = cvec_pool.tile([P, DC], F32, tag="rnk_part")
                nc.sync.dma_start(
                    rnk_part[:], nk_dram[0:1, :].rearrange("o (c p) -> (o p) c", p=P)
                )

                # A = relu(diag(rnk) Wk^T Mv)  (rnv deferred to output columns)
                for ekc in range(DC):
                    for eh in range(2):
                        pkv = psC_pool.tile([P, 512], F32, tag="psc", name="pkv")
                        for dc in range(DC):
                            nc.tensor.matmul(
                                pkv[:],
                                wk_sb[:, dc, ekc * P : (ekc + 1) * P],
                                mv_sb[:, dc, eh * 512 : (eh + 1) * 512],
                                start=(dc == 0),
                                stop=(dc == DC - 1),
                            )
                        nc.scalar.activation(
                            a_sb[:, ekc, eh * 512 : (eh + 1) * 512],
                            pkv[:],
                            RELU,
                            scale=rnk_part[:, ekc : ekc + 1],
                        )

                if _DEBUG_DUMP:
                    # rows 0:1024 G | 1024:2048 Mv | 2048:3072 A | 3072 rnv
                    # row 3073 rnk | rows 3074:4092 qT[0:1018, 0:1024]
                    yg = y_d[0:D, :].rearrange("(c p) e -> p c e", p=P)
                    nc.sync.dma_start(yg[:], g_sb[:].bitcast(F32))
                    ym = y_d[D : 2 * D, :].rearrange("(c p) e -> p c e", p=P)
                    nc.sync.dma_start(ym[:], mv_sb[:].bitcast(F32))
                    ya = y_d[2 * D : 3 * D, :].rearrange("(c p) e -> p c e", p=P)
                    nc.sync.dma_start(ya[:], a_sb[:].bitcast(F32))
                    nc.sync.dma_start(y_d[3 * D : 3 * D + 1, :], rnv_row[:])
                    nc.sync.dma_start(y_d[3 * D + 1 : 3 * D + 2, :], rnk_row[:])
                    nc.sync.dma_start(
                        y_d[3 * D + 2 : 4 * D - 2, :],
                        qT_dram[0 : D - 4, 0:D].bitcast(F32),
                    )

            # ---------------- Phase D: out = (query @ A) * rnv ----------------
            with (
                tc.tile_pool(name="qd", bufs=2) as qd_pool,
                tc.tile_pool(name="ot", bufs=3) as ot_pool,
                tc.tile_pool(name="po", bufs=4, space="PSUM") as po_pool,
            ):
                if _DEBUG_DUMP:
                    qd_pool, ot_pool, po_pool  # unused in debug builds
                else:
                    qT_r = qT_dram[:].rearrange("(c p) n -> p c n", p=P)  # [128, DC, N]
                    for s in range(NSLAB):
                        n0 = s * SLAB
                        qs = qd_pool.tile([P, DC, SLAB], F32R, tag="qs")
                        nc.sync.dma_start(qs[:], qT_r[:, :, n0 : n0 + SLAB])
                        for t in range(SLAB // P):
                            ot = ot_pool.tile([P, D], F32, tag="ot")
                            for eh in range(2):
                                po = po_pool.tile([P, 512], F32, tag="po", name="po")
                                for ec in range(DC):
                                    nc.tensor.matmul(
                                        po[:],
                                        qs[:, ec, t * P : (t + 1) * P],
                                        a_sb[:, ec, eh * 512 : (eh + 1) * 512],
                                        start=(ec == 0),
                                        stop=(ec == DC - 1),
                                    )
                                nc.vector.tensor_mul(
                                    ot[:, eh * 512 : (eh + 1) * 512],
                                    po[:],
                                    rnv_b[:, eh * 512 : (eh + 1) * 512],
                                )
                            nc.sync.dma_start(
                                y_d[n0 + t * P : n0 + (t + 1) * P, :], ot[:]
                            )

    _split_multi_waits(nc)
    return nc


_program_cache = None


def kernel(_trace=False, **inputs):
    global _program_cache, last_exec_time_ns, last_results
    if _program_cache is None:
        _program_cache = _build_program()
    nc = _program_cache

    x = np.ascontiguousarray(np.asarray(inputs["x"], dtype=np.float32))
    in_maps = []
    for b in range(B):
        in_maps.append(
            {
                "x": x[b],
                "w_query_real": np.asarray(inputs["w_query_real"], dtype=np.float32),
                "w_query_imag": np.asarray(inputs["w_query_imag"], dtype=np.float32),
                "w_key": np.asarray(inputs["w_key"], dtype=np.float32),
                "w_value": np.asarray(inputs["w_value"], dtype=np.float32),
            }
        )
    kwargs = {}
    if _trace:
        kwargs = dict(trace=True, tmpdir="/tmp/kernel_trace")
    res = run_bass_kernel_spmd(nc, in_maps, core_ids=list(range(B)), **kwargs)
    last_exec_time_ns = res.exec_time_ns
    last_results = res
    return np.stack([res.results[b]["y"] for b in range(B)], axis=0)



# revision 19
# speedup vs baseline: 1.0065x; 1.0065x over previous
"""Trainium2 Bass kernel for nn_BilinearFeedForward.

Math (per batch element b, reference semantics):
    q_r = x @ Wqr ; q_i = x @ Wqi ; query = relu(q_r) * relu(q_i)
    k = x @ Wk ; v = x @ Wv
    k /= (||k||_2 over n + eps) ; v /= (||v||_2 over n + eps)   (per column)
    kv = relu(k^T v)            [D, D]
    out = query @ kv            [N, D]

Key algebraic restructuring: with G = x^T x (symmetric, [D, D])
    k^T v       = Wk^T G Wv
    ||k_e||^2   = diag(Wk^T G Wk)_e ,  ||v_e||^2 = diag(Wv^T G Wv)_e
so k and v ([N, D] each) are never materialized; the sequence-length
reduction happens once inside G (upper-triangle blocks only, G symmetric).
rnk = 1/(nk+eps) folds into the relu-copy of KV as a per-partition ACT
scale (relu commutes with positive scaling); rnv = 1/(nv+eps) is a pure
column scaling of the final output.  query is produced transposed
(query^T, [D, N]) so the final einsum consumes it as the matmul stationary
operand directly.

All big matmuls run as float32r (full PE rate; fp32 is 4x slower).  The
BIR verifier requires f32r matmul inputs to be produced as f32r, so the
feeding DRAM tensors / SBUF tiles are declared float32r (same 4-byte bits).

Sharding: data-parallel over batch — 8 batch elements -> 8 NeuronCores,
weights replicated.  No collectives.
"""

import os

import numpy as np

import concourse.bass as bass
import concourse.mybir as mybir
import concourse.tile as tile
from concourse.bass_utils import run_bass_kernel_spmd
from concourse.masks import make_identity

F32 = mybir.dt.float32
F32R = mybir.dt.float32r
RELU = mybir.ActivationFunctionType.Relu
SQRT = mybir.ActivationFunctionType.Sqrt

B, N, D = 8, 4096, 1024
P = 128
DC = D // P          # 8 feature chunks
SLAB = 512           # token slab
NSLAB = N // SLAB    # 8
EPS = 1e-05

# G = x^T x upper-triangle piece tables: (rowblock i, colstart, width, bank, bankoff)
# Row-block i covers G[128i:128(i+1), 128i:1024]; pieces are matmul moving slices
# packed into [128, 512] f32 PSUM bank tiles.
G_PIECES_A = [  # row blocks 0..3 -> 7 banks (phase A)
    (0, 0, 512, 0, 0), (0, 512, 512, 1, 0),
    (1, 128, 512, 2, 0), (1, 640, 384, 3, 0),
    (3, 384, 128, 3, 384), (3, 512, 512, 4, 0),
    (2, 256, 512, 5, 0), (2, 768, 256, 6, 0),
]
N_BANKS_A = 7
G_PIECES_B = [  # row blocks 4..7 -> 3 banks (phase B)
    (4, 512, 512, 0, 0),
    (5, 640, 384, 1, 0), (7, 896, 128, 1, 384),
    (6, 768, 256, 2, 0),
]
N_BANKS_B = 3

last_exec_time_ns = None
last_results = None


def _split_multi_waits(nc, max_waits=1):
    """This container's walrus accepts at most ONE sync-wait per instruction
    ("Too many sync wait commands" otherwise).  Tile attaches several, so
    move the extras onto injected same-engine NoOps placed just before each
    offending instruction — engine streams dispatch in order, so a leading
    nop that blocks on the extra conditions is semantically identical."""
    ctr = 0
    for func in nc.m.functions:
        for bb in func.blocks:
            out = []
            changed = False
            for inst in bb.instructions:
                si = inst.sync_info
                waits = list(si.on_wait) if si and si.on_wait else []
                if len(waits) > max_waits:
                    for w in waits[:-max_waits]:
                        ctr += 1
                        nop = mybir.InstNoOp(
                            name=f"I-waitsplit-{ctr}",
                            engine=inst.engine,
                            sync_info=mybir.SyncInfo(on_wait=[w], on_update=[]),
                        )
                        nc.register_instruction(nop)
                        out.append(nop)
                    inst.sync_info = mybir.SyncInfo(
                        on_wait=waits[-max_waits:],
                        on_update=list(si.on_update) if si.on_update else [],
                    )
                    changed = True
                out.append(inst)
            if changed:
                bb.instructions = out
    return ctr


def _copy_r(nc, idx, out_r, in_ps):
    """PSUM -> f32r SBUF copy, alternating DVE / ACT.
    DVE path reads the source as f32r (f32r->f32r copy); ACT path reads it
    as f32 and casts on writeback (both verified on HW)."""
    if idx % 2 == 0:
        nc.vector.tensor_copy(out_r, in_ps.bitcast(F32R))
    else:
        nc.scalar.copy(out_r, in_ps.bitcast(F32))


def _load_weight(nc, pool, w_dram, name):
    """[D, D] f32r weight -> SBUF [128, DC, D] (partition = row-within-chunk),
    one DMA per row chunk so consumers can start before the full load."""
    t = pool.tile([P, DC, D], F32R, tag=name)
    wr = w_dram.rearrange("(c p) e -> p c e", p=P)
    for c in range(DC):
        nc.sync.dma_start(t[:, c, :], wr[:, c, :])
    return t


_DEBUG_DUMP = bool(os.environ.get("KERNEL_DEBUG_DUMP"))


def _build_program():
    # all data DMAs here are HWDGE (nc.sync); shrink the SWDGE descriptor-ring
    # SBUF carveout from its 16 KiB/partition default to reclaim SBUF
    nc = bass.Bass(dynamic_dma_scratch_size=2048)

    x_d = nc.dram_tensor("x", [N, D], F32R, kind="ExternalInput")
    wqr_d = nc.dram_tensor("w_query_real", [D, D], F32R, kind="ExternalInput")
    wqi_d = nc.dram_tensor("w_query_imag", [D, D], F32R, kind="ExternalInput")
    wk_d = nc.dram_tensor("w_key", [D, D], F32R, kind="ExternalInput")
    wv_d = nc.dram_tensor("w_value", [D, D], F32R, kind="ExternalInput")
    y_d = nc.dram_tensor("y", [N, D], F32, kind="ExternalOutput")

    x_r = x_d.rearrange("(s t p) d -> s p t d", p=P, t=SLAB // P)  # [8, 128, 4, 1024]

    with tile.TileContext(nc) as tc:
        with (
            tc.tile_pool(name="consts", bufs=1) as consts,
            tc.tile_pool(name="gsb", bufs=1) as gsb_pool,
            tc.tile_pool(name="asb", bufs=1) as a_pool,
            tc.tile_pool(name="vecs", bufs=1) as vecs_pool,
            tc.tile_pool(name="dram", bufs=1, space="DRAM") as dram_pool,
        ):
            ident_f = consts.tile([P, P], F32)
            make_identity(nc, ident_f)
            ident = consts.tile([P, P], F32R)
            nc.sync.dma_start(ident[:], ident_f[:].bitcast(F32R))
            ones = consts.tile([P, 1], F32)
            nc.vector.memset(ones, 1.0)

            g_sb = gsb_pool.tile([P, DC, D], F32R)  # full G, filled in pieces
            rnv_b = vecs_pool.tile([P, D], F32)     # 1/(nv+eps) bcast (phase D)
            qT_dram = dram_pool.tile([D, N], F32R)  # query^T spill

            with tc.tile_pool(name="wq", bufs=1) as wq_pool:
                # the two query-projection weights (used in phase B)
                wqr_sb = _load_weight(nc, wq_pool, wqr_d, "wqr")
                wqi_sb = _load_weight(nc, wq_pool, wqi_d, "wqi")

                # ---------------- Phase A: G row-blocks 0..3 ----------------
                with (
                    tc.tile_pool(name="xa", bufs=3) as xa_pool,
                    tc.tile_pool(name="psA", bufs=1, space="PSUM") as psA,
                ):
                    ga = [psA.tile([P, 512], F32, tag=f"ga{k}", name=f"ga{k}") for k in range(N_BANKS_A)]
                    for s in range(NSLAB):
                        xa = xa_pool.tile([P, SLAB // P, D], F32R, tag="xa")
                        nc.sync.dma_start(xa[:], x_r[s])
                        for t in range(SLAB // P):
                            for (i, cs, w, bk, off) in G_PIECES_A:
                                nc.tensor.matmul(
                                    ga[bk][:, off : off + w],
                                    xa[:, t, i * P : (i + 1) * P],
                                    xa[:, t, cs : cs + w],
                                    start=(s == 0 and t == 0),
                                    stop=(s == NSLAB - 1 and t == SLAB // P - 1),
                                )
                    # flush G rows 0..3 to SBUF
                    for n_, (i, cs, w, bk, off) in enumerate(G_PIECES_A):
                        _copy_r(nc, n_, g_sb[:, i, cs : cs + w], ga[bk][:, off : off + w])

                # ------- Phase B: transpose + query^T + G rows 4..7 ---------
                with (
                    tc.tile_pool(name="xb", bufs=2) as xb_pool,
                    tc.tile_pool(name="xt", bufs=2) as xt_pool,
                    tc.tile_pool(name="qre", bufs=2) as qre_pool,
                    tc.tile_pool(name="qto", bufs=3) as qto_pool,
                    tc.tile_pool(name="psB", bufs=1, space="PSUM") as psB,
                    tc.tile_pool(name="pt", bufs=2, space="PSUM") as pt_pool,
                    tc.tile_pool(name="pq", bufs=3, space="PSUM") as pq_pool,
                ):
                    gb = [psB.tile([P, 512], F32, tag=f"gb{k}", name=f"gb{k}") for k in range(N_BANKS_B)]
                    for s in range(NSLAB):
                        n0 = s * SLAB
                        xb = xb_pool.tile([P, SLAB // P, D], F32R, tag="xb")
                        nc.sync.dma_start(xb[:], x_r[s])

                        # transpose x slab -> x^T slab [128(d), DC, SLAB]
                        xt = xt_pool.tile([P, DC, SLAB], F32R, tag="xt")
                        for dc in range(DC):
                            ptile = pt_pool.tile([P, SLAB], F32R, tag="pt", name="pt")
                            for t in range(SLAB // P):
                                nc.tensor.transpose(
                                    ptile[:, t * P : (t + 1) * P],
                                    xb[:, t, dc * P : (dc + 1) * P],
                                    ident,
                                )
                            _copy_r(nc, dc, xt[:, dc, :], ptile[:].bitcast(F32))

                        # G row-blocks 4..7 accumulation
                        for t in range(SLAB // P):
                            for (i, cs, w, bk, off) in G_PIECES_B:
                                nc.tensor.matmul(
                                    gb[bk][:, off : off + w],
                                    xb[:, t, i * P : (i + 1) * P],
                                    xb[:, t, cs : cs + w],
                                    start=(s == 0 and t == 0),
                                    stop=(s == NSLAB - 1 and t == SLAB // P - 1),
                                )

                        # query^T = relu(Wqr^T x^T) * relu(Wqi^T x^T)
                        for ec in range(DC):
                            pr = pq_pool.tile([P, SLAB], F32, tag="pq", name="pr")
                            for dc in range(DC):
                                nc.tensor.matmul(
                                    pr[:],
                                    wqr_sb[:, dc, ec * P : (ec + 1) * P],
                                    xt[:, dc, :],
                                    start=(dc == 0),
                                    stop=(dc == DC - 1),
                                )
                            pi = pq_pool.tile([P, SLAB], F32, tag="pq", name="pi")
                            for dc in range(DC):
                                nc.tensor.matmul(
                                    pi[:],
                                    wqi_sb[:, dc, ec * P : (ec + 1) * P],
                                    xt[:, dc, :],
                                    start=(dc == 0),
                                    stop=(dc == DC - 1),
                                )
                            rr = qre_pool.tile([P, SLAB], F32, tag="rr")
                            nc.scalar.activation(rr[:], pr[:], RELU)
                            ri = qre_pool.tile([P, SLAB], F32, tag="ri")
                            nc.scalar.activation(ri[:], pi[:], RELU)
                            qt = qto_pool.tile([P, SLAB], F32, tag="qt")
                            nc.vector.tensor_mul(qt[:], rr[:], ri[:])
                            nc.sync.dma_start(
                                qT_dram[ec * P : (ec + 1) * P, n0 : n0 + SLAB],
                                qt[:].bitcast(F32R),
                            )
                    # flush G rows 4..7
                    for n_, (i, cs, w, bk, off) in enumerate(G_PIECES_B):
                        _copy_r(nc, n_, g_sb[:, i, cs : cs + w], gb[bk][:, off : off + w])

            # mirror the strictly-upper blocks of G into the lower triangle
            with tc.tile_pool(name="ptC", bufs=2, space="PSUM") as ptC_pool:
                for i in range(DC):
                    for j in range(i + 1, DC):
                        ptile = ptC_pool.tile([P, P], F32R, tag="ptc", name="ptc")
                        nc.tensor.transpose(
                            ptile[:], g_sb[:, i, j * P : (j + 1) * P], ident
                        )
                        _copy_r(nc, i + j, g_sb[:, j, i * P : (i + 1) * P],
                                ptile[:].bitcast(F32))

            # ---------------- Phase C: KV attention matrix A ----------------
            with (
                tc.tile_pool(name="wkv", bufs=1) as wkv_pool,
                tc.tile_pool(name="mv", bufs=1) as mv_pool,
                tc.tile_pool(name="ctmp", bufs=2) as ctmp_pool,
                tc.tile_pool(name="cvec", bufs=1) as cvec_pool,
                tc.tile_pool(name="psC", bufs=3, space="PSUM") as psC_pool,
                tc.tile_pool(name="pnrm", bufs=1, space="PSUM") as pnrm_pool,
            ):
                a_sb = a_pool.tile([P, DC, D], F32R)  # relu'd row-scaled KV
                wk_sb = _load_weight(nc, wkv_pool, wk_d, "wk")
                wv_sb = _load_weight(nc, wkv_pool, wv_d, "wv")
                mv_sb = mv_pool.tile([P, DC, D], F32R)

                # Mv = G Wv   [D, D]
                for mc in range(DC):
                    for eh in range(2):
                        pm = psC_pool.tile([P, 512], F32, tag="psc", name="pm")
                        for dc in range(DC):
                            nc.tensor.matmul(
                                pm[:],
                                g_sb[:, dc, mc * P : (mc + 1) * P],
                                wv_sb[:, dc, eh * 512 : (eh + 1) * 512],
                                start=(dc == 0),
                                stop=(dc == DC - 1),
                            )
                        _copy_r(nc, eh, mv_sb[:, mc, eh * 512 : (eh + 1) * 512], pm[:])

                # nv^2 = colsum(Wv * Mv) ; rnv = 1/(sqrt(nv^2)+eps)
                pnv = [pnrm_pool.tile([1, 512], F32, tag=f"pnv{h}", name=f"pnv{h}") for h in range(2)]
                for mc in range(DC):
                    tmpv = ctmp_pool.tile([P, D], F32, tag="tmpv")
                    nc.vector.tensor_mul(
                        tmpv[:],
                        wv_sb[:, mc, :].bitcast(F32),
                        mv_sb[:, mc, :].bitcast(F32),
                    )
                    for eh in range(2):
                        nc.tensor.matmul(
                            pnv[eh][:],
                            ones[:, 0:1],
                            tmpv[:, eh * 512 : (eh + 1) * 512],
                            start=(mc == 0),
                            stop=(mc == DC - 1),
                        )
                rnv_row = cvec_pool.tile([1, D], F32, tag="rnv_row")
                for eh in range(2):
                    nc.scalar.activation(
                        rnv_row[:, eh * 512 : (eh + 1) * 512], pnv[eh][:], SQRT
                    )
                nc.vector.tensor_scalar_add(rnv_row[:], rnv_row[:], EPS)
                nc.vector.reciprocal(rnv_row[:], rnv_row[:])
                # broadcast [1, D] across partitions via DRAM bounce
                nv_dram = dram_pool.tile([1, D], F32)
                nc.sync.dma_start(nv_dram[:], rnv_row[:])
                nc.sync.dma_start(rnv_b[:], nv_dram[0:1, :].to_broadcast((P, D)))

                # nk^2 via Mk = G Wk (not materialized) ; rnk
                pnk = [pnrm_pool.tile([1, 512], F32, tag=f"pnk{h}", name=f"pnk{h}") for h in range(2)]
                for mc in range(DC):
                    tmpk = ctmp_pool.tile([P, D], F32, tag="tmpk")
                    for eh in range(2):
                        pk = psC_pool.tile([P, 512], F32, tag="psc", name="pk")
                        for dc in range(DC):
                            nc.tensor.matmul(
                                pk[:],
                                g_sb[:, dc, mc * P : (mc + 1) * P],
                                wk_sb[:, dc, eh * 512 : (eh + 1) * 512],
                                start=(dc == 0),
                                stop=(dc == DC - 1),
                            )
                        nc.vector.tensor_mul(
                            tmpk[:, eh * 512 : (eh + 1) * 512],
                            wk_sb[:, mc, eh * 512 : (eh + 1) * 512].bitcast(F32),
                            pk[:],
                        )
                    for eh in range(2):
                        nc.tensor.matmul(
                            pnk[eh][:],
                            ones[:, 0:1],
                            tmpk[:, eh * 512 : (eh + 1) * 512],
                            start=(mc == 0),
                            stop=(mc == DC - 1),
                        )
                rnk_row = cvec_pool.tile([1, D], F32, tag="rnk_row")
                for eh in range(2):
                    nc.scalar.activation(
                        rnk_row[:, eh * 512 : (eh + 1) * 512], pnk[eh][:], SQRT
                    )
                nc.vector.tensor_scalar_add(rnk_row[:], rnk_row[:], EPS)
                nc.vector.reciprocal(rnk_row[:], rnk_row[:])
                nk_dram = dram_pool.tile([1, D], F32)
                nc.sync.dma_start(nk_dram[:], rnk_row[:])
                # rnk in per-partition layout [128, DC]: partition p <-> e_k = ec*128+p
                rnk_part = cvec_pool.tile([P, DC], F32, tag="rnk_part")
                nc.sync.dma_start(
                    rnk_part[:], nk_dram[0:1, :].rearrange("o (c p) -> (o p) c", p=P)
                )

                # A = relu(diag(rnk) Wk^T Mv)  (rnv deferred to output columns)
                for ekc in range(DC):
                    for eh in range(2):
                        pkv = psC_pool.tile([P, 512], F32, tag="psc", name="pkv")
                        for dc in range(DC):
                            nc.tensor.matmul(
                                pkv[:],
                                wk_sb[:, dc, ekc * P : (ekc + 1) * P],
                                mv_sb[:, dc, eh * 512 : (eh + 1) * 512],
                                start=(dc == 0),
                                stop=(dc == DC - 1),
                            )
                        nc.scalar.activation(
                            a_sb[:, ekc, eh * 512 : (eh + 1) * 512],
                            pkv[:],
                            RELU,
                            scale=rnk_part[:, ekc : ekc + 1],
                        )

                if _DEBUG_DUMP:
                    # rows 0:1024 G | 1024:2048 Mv | 2048:3072 A | 3072 rnv
                    # row 3073 rnk | rows 3074:4092 qT[0:1018, 0:1024]
                    yg = y_d[0:D, :].rearrange("(c p) e -> p c e", p=P)
                    nc.sync.dma_start(yg[:], g_sb[:].bitcast(F32))
                    ym = y_d[D : 2 * D, :].rearrange("(c p) e -> p c e", p=P)
                    nc.sync.dma_start(ym[:], mv_sb[:].bitcast(F32))
                    ya = y_d[2 * D : 3 * D, :].rearrange("(c p) e -> p c e", p=P)
                    nc.sync.dma_start(ya[:], a_sb[:].bitcast(F32))
                    nc.sync.dma_start(y_d[3 * D : 3 * D + 1, :], rnv_row[:])
                    nc.sync.dma_start(y_d[3 * D + 1 : 3 * D + 2, :], rnk_row[:])
                    nc.sync.dma_start(
                        y_d[3 * D + 2 : 4 * D - 2, :],
                        qT_dram[0 : D - 4, 0:D].bitcast(F32),
                    )

            # ---------------- Phase D: out = (query @ A) * rnv ----------------
            with (
                tc.tile_pool(name="qd", bufs=2) as qd_pool,
                tc.tile_pool(name="ot", bufs=3) as ot_pool,
                tc.tile_pool(name="po", bufs=4, space="PSUM") as po_pool,
            ):
                if _DEBUG_DUMP:
                    qd_pool, ot_pool, po_pool  # unused in debug builds
                else:
                    qT_r = qT_dram[:].rearrange("(c p) n -> p c n", p=P)  # [128, DC, N]
                    for s in range(NSLAB):
                        n0 = s * SLAB
                        qs = qd_pool.tile([P, DC, SLAB], F32R, tag="qs")
                        nc.sync.dma_start(qs[:], qT_r[:, :, n0 : n0 + SLAB])
                        for t in range(SLAB // P):
                            ot = ot_pool.tile([P, D], F32, tag="ot")
                            for eh in range(2):
                                po = po_pool.tile([P, 512], F32, tag="po", name="po")
                                for ec in range(DC):
                                    nc.tensor.matmul(
                                        po[:],
                                        qs[:, ec, t * P : (t + 1) * P],
                                        a_sb[:, ec, eh * 512 : (eh + 1) * 512],
                                        start=(ec == 0),
                                        stop=(ec == DC - 1),
                                    )
                                nc.vector.tensor_mul(
                                    ot[:, eh * 512 : (eh + 1) * 512],
                                    po[:],
                                    rnv_b[:, eh * 512 : (eh + 1) * 512],
                                )
                            nc.sync.dma_start(
                                y_d[n0 + t * P : n0 + (t + 1) * P, :], ot[:]
                            )

    _split_multi_waits(nc)
    return nc


_program_cache = None


def kernel(_trace=False, **inputs):
    global _program_cache, last_exec_time_ns, last_results
    if _program_cache is None:
        _program_cache = _build_program()
    nc = _program_cache

    x = np.ascontiguousarray(np.asarray(inputs["x"], dtype=np.float32))
    in_maps = []
    for b in range(B):
        in_maps.append(
            {
                "x": x[b],
                "w_query_real": np.asarray(inputs["w_query_real"], dtype=np.float32),
                "w_query_imag": np.asarray(inputs["w_query_imag"], dtype=np.float32),
                "w_key": np.asarray(inputs["w_key"], dtype=np.float32),
                "w_value": np.asarray(inputs["w_value"], dtype=np.float32),
            }
        )
    kwargs = {}
    if _trace:
        kwargs = dict(trace=True, tmpdir="/tmp/kernel_trace")
    res = run_bass_kernel_spmd(nc, in_maps, core_ids=list(range(B)), **kwargs)
    last_exec_time_ns = res.exec_time_ns
    last_results = res
    return np.stack([res.results[b]["y"] for b in range(B)], axis=0)



# revision 20
# speedup vs baseline: 1.1876x; 1.1799x over previous
"""Trainium2 Bass kernel for nn_BilinearFeedForward.

Math (per batch element b, reference semantics):
    q_r = x @ Wqr ; q_i = x @ Wqi ; query = relu(q_r) * relu(q_i)
    k = x @ Wk ; v = x @ Wv
    k /= (||k||_2 over n + eps) ; v /= (||v||_2 over n + eps)   (per column)
    kv = relu(k^T v)            [D, D]
    out = query @ kv            [N, D]

Algebraic restructuring: with G = x^T x (symmetric, [D, D])
    k^T v       = Wk^T G Wv
    ||k_e||^2   = diag(Wk^T G Wk)_e ,  ||v_e||^2 = diag(Wv^T G Wv)_e
so k and v are never materialized.  rnk = 1/(nk+eps) folds into the
stationary operand of the KV matmul (Wk columns pre-scaled); rnv is a pure
column scaling of the final output.

v2 structure (single fused pass over x):
  - x is loaded ONCE; per 512-token slab the kernel (a) accumulates the
    upper-triangle pieces of G in rotating PSUM banks and adds them into an
    SBUF f32 accumulator (DVE), (b) transposes the slab (bf16) for the
    query projections, (c) runs the query matmuls for the PREVIOUS slab
    (lag-1 software pipeline so the weight DMAs are off the critical path).
  - query path runs in bf16 (x-transpose and Wq cast to bf16): same PE rate
    as f32r but half the SBUF/DMA bytes; query^T spills to DRAM as bf16.
  - all G pieces have moving width >= 256 (narrow f32r matmuls run at 1/4
    rate); the row-7 piece is widened to 256 into the lower triangle, so
    mirror (6,7) is skipped.
  - norms use an all-ones [128,128] stationary so the column sums land
    broadcast across all partitions - no DRAM bounce, no gather.
  - phase C computes the nk path FIRST so its serial sqrt/recip chain hides
    under the Mv matmuls; colsum matmuls are software-pipelined by one step
    so the PE never waits on the DVE elementwise products.
  - phase D consumes query^T (bf16, prefetched) against A (bf16) and scales
    by rnv broadcast.

Sharding: data-parallel over batch - 8 batch elements -> 8 NeuronCores,
weights replicated.  No collectives.
"""

import numpy as np

import concourse.bass as bass
import concourse.mybir as mybir
import concourse.tile as tile
from concourse.bass_utils import run_bass_kernel_spmd
from concourse.masks import make_identity

F32 = mybir.dt.float32
F32R = mybir.dt.float32r
BF16 = mybir.dt.bfloat16
RELU = mybir.ActivationFunctionType.Relu
SQRT = mybir.ActivationFunctionType.Sqrt

B, N, D = 8, 4096, 1024
P = 128
DC = D // P          # 8 feature chunks
SLAB = 512           # token slab
TPS = SLAB // P      # 4 token tiles per slab
NSLAB = N // SLAB    # 8
EPS = 1e-05

# G = x^T x upper-triangle pieces (rowblock i, colstart, width).
# All widths >= 256 so f32r matmuls run at full rate.  Row 7's piece is
# widened to [768:1024) (computes lower block (7,6) redundantly), so the
# (6,7) mirror transpose is skipped.
G_PIECES = [
    (0, 0, 512), (0, 512, 512),
    (1, 128, 512), (1, 640, 384),
    (2, 256, 512), (2, 768, 256),
    (3, 384, 384), (3, 768, 256),
    (4, 512, 512),
    (5, 640, 384),
    (6, 768, 256),
    (7, 768, 256),
]

last_exec_time_ns = None
last_results = None


def _split_multi_waits(nc, max_waits=1):
    """This container's walrus accepts at most ONE sync-wait per instruction
    ("Too many sync wait commands" otherwise).  Tile attaches several, so
    move the extras onto injected same-engine NoOps placed just before each
    offending instruction - engine streams dispatch in order, so a leading
    nop that blocks on the extra conditions is semantically identical."""
    ctr = 0
    for func in nc.m.functions:
        for bb in func.blocks:
            out = []
            changed = False
            for inst in bb.instructions:
                si = inst.sync_info
                waits = list(si.on_wait) if si and si.on_wait else []
                if len(waits) > max_waits:
                    for w in waits[:-max_waits]:
                        ctr += 1
                        nop = mybir.InstNoOp(
                            name=f"I-waitsplit-{ctr}",
                            engine=inst.engine,
                            sync_info=mybir.SyncInfo(on_wait=[w], on_update=[]),
                        )
                        nc.register_instruction(nop)
                        out.append(nop)
                    inst.sync_info = mybir.SyncInfo(
                        on_wait=waits[-max_waits:],
                        on_update=list(si.on_update) if si.on_update else [],
                    )
                    changed = True
                out.append(inst)
            if changed:
                bb.instructions = out
    return ctr


def _copy_r(nc, idx, out_r, in_ps):
    """PSUM -> f32r SBUF copy, alternating DVE / ACT."""
    if idx % 2 == 0:
        nc.vector.tensor_copy(out_r, in_ps.bitcast(F32R))
    else:
        nc.scalar.copy(out_r, in_ps.bitcast(F32))


def _build_program():
    nc = bass.Bass(dynamic_dma_scratch_size=2048)

    x_d = nc.dram_tensor("x", [N, D], F32R, kind="ExternalInput")
    wqr_d = nc.dram_tensor("w_query_real", [D, D], F32R, kind="ExternalInput")
    wqi_d = nc.dram_tensor("w_query_imag", [D, D], F32R, kind="ExternalInput")
    wk_d = nc.dram_tensor("w_key", [D, D], F32R, kind="ExternalInput")
    wv_d = nc.dram_tensor("w_value", [D, D], F32R, kind="ExternalInput")
    y_d = nc.dram_tensor("y", [N, D], F32, kind="ExternalOutput")

    x_r = x_d.rearrange("(s t p) d -> s p t d", p=P, t=TPS)  # [8, 128, 4, 1024]

    with tile.TileContext(nc) as tc:
        with (
            tc.tile_pool(name="consts", bufs=1) as consts,
            tc.tile_pool(name="gsb", bufs=1) as gsb_pool,
            tc.tile_pool(name="absb", bufs=1) as a_pool,
            tc.tile_pool(name="vecs", bufs=1) as vecs_pool,
            tc.tile_pool(name="dram", bufs=1, space="DRAM") as dram_pool,
        ):
            ident_f = consts.tile([P, P], F32)
            make_identity(nc, ident_f)
            ident = consts.tile([P, P], F32R)
            nc.sync.dma_start(ident[:], ident_f[:].bitcast(F32R))
            ident16 = consts.tile([P, P], BF16)
            make_identity(nc, ident16)
            ones_f = consts.tile([P, P], F32)
            nc.vector.memset(ones_f, 1.0)
            ones = consts.tile([P, P], F32R)
            nc.sync.dma_start(ones[:], ones_f[:].bitcast(F32R))

            g_sb = gsb_pool.tile([P, DC, D], F32R)  # G accumulator
            rnv_b = vecs_pool.tile([P, D], F32)     # 1/(nv+eps), bcast rows
            qT_dram = dram_pool.tile([D, N], BF16)  # query^T spill (bf16)

            with tc.tile_pool(name="wkv", bufs=1) as wkv_pool:
                # wk/wv tiles are allocated late (wk at slab 3, wv at phase
                # C) to stay under the SBUF budget during the fused pass.
                wk_sb = None
                wv_sb = None
                wkr = wk_d.rearrange("(c p) e -> p c e", p=P)
                wvr = wv_d.rearrange("(c p) e -> p c e", p=P)

                # ================= fused pass over x =================
                with (
                    tc.tile_pool(name="wq16", bufs=1) as wq16_pool,
                    tc.tile_pool(name="xa", bufs=2) as xa_pool,
                    tc.tile_pool(name="xa16", bufs=2) as xa16_pool,
                    tc.tile_pool(name="xt", bufs=2) as xt_pool,
                    tc.tile_pool(name="rr", bufs=1) as rr_pool,
                    tc.tile_pool(name="ri", bufs=2) as ri_pool,
                    tc.tile_pool(name="qt", bufs=2) as qt_pool,
                    tc.tile_pool(name="gps", bufs=2, space="PSUM") as gps_pool,
                    tc.tile_pool(name="pt", bufs=2, space="PSUM") as pt_pool,
                    tc.tile_pool(name="pq", bufs=3, space="PSUM") as pq_pool,
                ):
                    # x slabs 0/1 first so the PE can start immediately;
                    # query weights right behind (needed by Q(0) ~30us in).
                    xa_tiles = [None] * NSLAB
                    for s in (0, 1):
                        xa_tiles[s] = xa_pool.tile([P, TPS, D], F32R, tag="xa", name="xa")
                        nc.sync.dma_start(xa_tiles[s][:], x_r[s])

                    wqr16 = wq16_pool.tile([P, DC, D], BF16, tag="wqr")
                    wqi16 = wq16_pool.tile([P, DC, D], BF16, tag="wqi")
                    with tc.tile_pool(name="wstg", bufs=2) as wstg_pool:
                        for w_dram, w16 in ((wqr_d, wqr16), (wqi_d, wqi16)):
                            wr = w_dram.rearrange("(c p) e -> p c e", p=P)
                            for c in range(DC):
                                stg = wstg_pool.tile([P, D], F32R, tag="wstg")
                                nc.sync.dma_start(stg[:], wr[:, c, :])
                                nc.scalar.copy(w16[:, c, :], stg[:].bitcast(F32))

                    xt_tiles = [None, None]  # rotating per-slab x^T (bf16)

                    def emit_q(sq):
                        """query^T for slab sq: all q_r chains first (so the
                        wqi DMA can still be in flight), then q_i + combine."""
                        n0 = sq * SLAB
                        xt = xt_tiles[sq % 2]
                        rr16 = rr_pool.tile([P, DC, SLAB], BF16, tag="rr")
                        for ec in range(DC):
                            pr = pq_pool.tile([P, SLAB], F32, tag="pq")
                            for dc in range(DC):
                                nc.tensor.matmul(
                                    pr[:],
                                    wqr16[:, dc, ec * P : (ec + 1) * P],
                                    xt[:, dc, :],
                                    start=(dc == 0),
                                    stop=(dc == DC - 1),
                                )
                            nc.scalar.activation(rr16[:, ec, :], pr[:], RELU)
                        for ec in range(DC):
                            pi = pq_pool.tile([P, SLAB], F32, tag="pq")
                            for dc in range(DC):
                                nc.tensor.matmul(
                                    pi[:],
                                    wqi16[:, dc, ec * P : (ec + 1) * P],
                                    xt[:, dc, :],
                                    start=(dc == 0),
                                    stop=(dc == DC - 1),
                                )
                            ri16 = ri_pool.tile([P, SLAB], BF16, tag="ri")
                            nc.scalar.activation(ri16[:], pi[:], RELU)
                            qt16 = qt_pool.tile([P, SLAB], BF16, tag="qt")
                            nc.vector.tensor_mul(qt16[:], rr16[:, ec, :], ri16[:])
                            nc.sync.dma_start(
                                qT_dram[ec * P : (ec + 1) * P, n0 : n0 + SLAB],
                                qt16[:],
                            )

                    for s in range(NSLAB):
                        if s == 3:
                            wk_sb = wkv_pool.tile([P, DC, D], F32R, tag="wk", name="wk_sb")
                            for c in range(DC):
                                nc.sync.dma_start(wk_sb[:, c, :], wkr[:, c, :])

                        xa = xa_tiles[s]
                        # bf16 cast of the slab (ACT), one instr per token tile
                        xa16 = xa16_pool.tile([P, TPS, D], BF16, tag="xa16")
                        for t in range(TPS):
                            nc.scalar.copy(xa16[:, t, :], xa[:, t, :].bitcast(F32))

                        # G pieces: accumulate over the slab's 4 token tiles
                        # in PSUM, then add into the SBUF accumulator.
                        for pidx, (i, cs, w) in enumerate(G_PIECES):
                            gps = gps_pool.tile([P, 512], F32, tag="gps")
                            for t in range(TPS):
                                nc.tensor.matmul(
                                    gps[:, :w],
                                    xa[:, t, i * P : (i + 1) * P],
                                    xa[:, t, cs : cs + w],
                                    start=(t == 0),
                                    stop=(t == TPS - 1),
                                )
                            dst = g_sb[:, i, cs : cs + w]
                            if s == 0:
                                nc.vector.tensor_copy(dst, gps[:, :w].bitcast(F32R))
                            else:
                                nc.vector.tensor_add(dst, gps[:, :w].bitcast(F32R), dst)

                        # transpose slab -> x^T (bf16) [128(d), DC, SLAB]
                        xt = xt_pool.tile([P, DC, SLAB], BF16, tag="xt")
                        xt_tiles[s % 2] = xt
                        for dc in range(DC):
                            ptile = pt_pool.tile([P, SLAB], BF16, tag="pt")
                            for t in range(TPS):
                                nc.tensor.transpose(
                                    ptile[:, t * P : (t + 1) * P],
                                    xa16[:, t, dc * P : (dc + 1) * P],
                                    ident16,
                                )
                            nc.vector.tensor_copy(xt[:, dc, :], ptile[:])

                        # prefetch the next-but-one slab; emitted after this
                        # slab's reads so the queue-head wait is short
                        if s + 2 < NSLAB:
                            xa_tiles[s + 2] = xa_pool.tile([P, TPS, D], F32R, tag="xa", name="xa")
                            nc.sync.dma_start(xa_tiles[s + 2][:], x_r[s + 2])

                        if s > 0:
                            emit_q(s - 1)
                    emit_q(NSLAB - 1)

                # mirror strictly-upper blocks of G into the lower triangle
                with tc.tile_pool(name="ptC", bufs=2, space="PSUM") as ptC_pool:
                    nmir = 0
                    for i in range(DC):
                        for j in range(i + 1, DC):
                            if (i, j) == (6, 7):
                                continue  # computed directly by row-7 piece
                            ptile = ptC_pool.tile([P, P], F32R, tag="ptc")
                            nc.tensor.transpose(
                                ptile[:], g_sb[:, i, j * P : (j + 1) * P], ident
                            )
                            _copy_r(nc, nmir, g_sb[:, j, i * P : (i + 1) * P],
                                    ptile[:].bitcast(F32))
                            nmir += 1

                # ================= phase C: norms + A =================
                with (
                    tc.tile_pool(name="mv", bufs=1) as mv_pool,
                    tc.tile_pool(name="wks", bufs=1) as wks_pool,
                    tc.tile_pool(name="cvec", bufs=1) as cvec_pool,
                    tc.tile_pool(name="ctmp", bufs=2) as ctmp_pool,
                    tc.tile_pool(name="psC", bufs=3, space="PSUM") as psC_pool,
                    tc.tile_pool(name="pn", bufs=1, space="PSUM") as pn_pool,
                ):
                    wv_sb = wkv_pool.tile([P, DC, D], F32R, tag="wv", name="wv_sb")
                    for c in range(DC):
                        nc.sync.dma_start(wv_sb[:, c, :], wvr[:, c, :])
                    a16 = a_pool.tile([P, DC, D], BF16, name="a16")
                    mv_sb = mv_pool.tile([P, DC, D], F32R)
                    wks_sb = wks_pool.tile([P, DC, D], F32R)
                    rnk_b = cvec_pool.tile([P, D], F32, tag="rnk")

                    # ---- nk path: Mk = G Wk (not materialized), colsums
                    # land broadcast via all-ones stationary.  The colsum
                    # matmul for step k is emitted during step k+1 so the PE
                    # never waits on the DVE product.
                    pnk = [pn_pool.tile([P, 512], F32, tag=f"pnk{h}", name=f"pnk{h}") for h in range(2)]
                    pend = None  # (tmpk tile, eh, start, stop)
                    for mc in range(DC):
                        for eh in range(2):
                            pk = psC_pool.tile([P, 512], F32, tag="psc")
                            for dc in range(DC):
                                nc.tensor.matmul(
                                    pk[:],
                                    g_sb[:, dc, mc * P : (mc + 1) * P],
                                    wk_sb[:, dc, eh * 512 : (eh + 1) * 512],
                                    start=(dc == 0),
                                    stop=(dc == DC - 1),
                                )
                            if pend is not None:
                                tp, teh, tst, tsp = pend
                                nc.tensor.matmul(pnk[teh][:], ones[:], tp[:],
                                                 start=tst, stop=tsp)
                            tmpk = ctmp_pool.tile([P, 512], F32R, tag="tmpk")
                            nc.vector.tensor_mul(
                                tmpk[:],
                                wk_sb[:, mc, eh * 512 : (eh + 1) * 512],
                                pk[:].bitcast(F32R),
                            )
                            pend = (tmpk, eh, mc == 0, mc == DC - 1)
                    tp, teh, tst, tsp = pend
                    nc.tensor.matmul(pnk[teh][:], ones[:], tp[:], start=tst, stop=tsp)

                    # rnk = 1/(sqrt(nk^2)+eps)  (broadcast on all partitions)
                    for eh in range(2):
                        nc.scalar.activation(
                            rnk_b[:, eh * 512 : (eh + 1) * 512], pnk[eh][:], SQRT
                        )
                    nc.vector.tensor_scalar_add(rnk_b[:], rnk_b[:], EPS)
                    nc.vector.reciprocal(rnk_b[:], rnk_b[:])

                    # ---- nv path + Mv materialization
                    pnv = [pn_pool.tile([P, 512], F32, tag=f"pnv{h}", name=f"pnv{h}") for h in range(2)]
                    pend = None
                    cc = 0
                    for mc in range(DC):
                        for eh in range(2):
                            pm = psC_pool.tile([P, 512], F32, tag="psc")
                            for dc in range(DC):
                                nc.tensor.matmul(
                                    pm[:],
                                    g_sb[:, dc, mc * P : (mc + 1) * P],
                                    wv_sb[:, dc, eh * 512 : (eh + 1) * 512],
                                    start=(dc == 0),
                                    stop=(dc == DC - 1),
                                )
                            if pend is not None:
                                tp, teh, tst, tsp = pend
                                nc.tensor.matmul(pnv[teh][:], ones[:], tp[:],
                                                 start=tst, stop=tsp)
                            _copy_r(nc, cc, mv_sb[:, mc, eh * 512 : (eh + 1) * 512], pm[:])
                            cc += 1
                            tmpv = ctmp_pool.tile([P, 512], F32R, tag="tmpv")
                            nc.vector.tensor_mul(
                                tmpv[:],
                                wv_sb[:, mc, eh * 512 : (eh + 1) * 512],
                                pm[:].bitcast(F32R),
                            )
                            pend = (tmpv, eh, mc == 0, mc == DC - 1)
                    tp, teh, tst, tsp = pend
                    nc.tensor.matmul(pnv[teh][:], ones[:], tp[:], start=tst, stop=tsp)

                    # wks = Wk * rnk (column scale of the A stationary); DVE
                    # work hides under the Mv matmuls above.
                    for mc in range(DC):
                        nc.vector.tensor_mul(
                            wks_sb[:, mc, :],
                            wk_sb[:, mc, :],
                            rnk_b[:].bitcast(F32R),
                        )

                    # rnv = 1/(sqrt(nv^2)+eps)
                    for eh in range(2):
                        nc.scalar.activation(
                            rnv_b[:, eh * 512 : (eh + 1) * 512], pnv[eh][:], SQRT
                        )
                    nc.vector.tensor_scalar_add(rnv_b[:], rnv_b[:], EPS)
                    nc.vector.reciprocal(rnv_b[:], rnv_b[:])

                    # ---- A = relu(diag(rnk) Wk^T Mv)  -> bf16
                    for ekc in range(DC):
                        for eh in range(2):
                            pkv = psC_pool.tile([P, 512], F32, tag="psc")
                            for dc in range(DC):
                                nc.tensor.matmul(
                                    pkv[:],
                                    wks_sb[:, dc, ekc * P : (ekc + 1) * P],
                                    mv_sb[:, dc, eh * 512 : (eh + 1) * 512],
                                    start=(dc == 0),
                                    stop=(dc == DC - 1),
                                )
                            nc.scalar.activation(
                                a16[:, ekc, eh * 512 : (eh + 1) * 512], pkv[:], RELU
                            )

            # ============= phase D: out = (query @ A) * rnv =============
            with (
                tc.tile_pool(name="qd", bufs=3) as qd_pool,
                tc.tile_pool(name="ot", bufs=3) as ot_pool,
                tc.tile_pool(name="po", bufs=3, space="PSUM") as po_pool,
            ):
                qT_r = qT_dram[:].rearrange("(c p) n -> p c n", p=P)
                qs_tiles = [None] * NSLAB
                for s in (0, 1):
                    qs_tiles[s] = qd_pool.tile([P, DC, SLAB], BF16, tag="qs", name="qs")
                    nc.sync.dma_start(
                        qs_tiles[s][:], qT_r[:, :, s * SLAB : (s + 1) * SLAB]
                    )
                for s in range(NSLAB):
                    n0 = s * SLAB
                    if s + 2 < NSLAB:
                        qs_tiles[s + 2] = qd_pool.tile(
                            [P, DC, SLAB], BF16, tag="qs", name="qs"
                        )
                        nc.sync.dma_start(
                            qs_tiles[s + 2][:],
                            qT_r[:, :, (s + 2) * SLAB : (s + 3) * SLAB],
                        )
                    qs = qs_tiles[s]
                    for t in range(TPS):
                        ot = ot_pool.tile([P, D], F32, tag="ot")
                        for eh in range(2):
                            po = po_pool.tile([P, 512], F32, tag="po")
                            for ec in range(DC):
                                nc.tensor.matmul(
                                    po[:],
                                    qs[:, ec, t * P : (t + 1) * P],
                                    a16[:, ec, eh * 512 : (eh + 1) * 512],
                                    start=(ec == 0),
                                    stop=(ec == DC - 1),
                                )
                            nc.vector.tensor_mul(
                                ot[:, eh * 512 : (eh + 1) * 512],
                                po[:],
                                rnv_b[:, eh * 512 : (eh + 1) * 512],
                            )
                        nc.sync.dma_start(
                            y_d[n0 + t * P : n0 + (t + 1) * P, :], ot[:]
                        )

    _split_multi_waits(nc)
    return nc


_program_cache = None


def kernel(_trace=False, **inputs):
    global _program_cache, last_exec_time_ns, last_results
    if _program_cache is None:
        _program_cache = _build_program()
    nc = _program_cache

    x = np.ascontiguousarray(np.asarray(inputs["x"], dtype=np.float32))
    in_maps = []
    for b in range(B):
        in_maps.append(
            {
                "x": x[b],
                "w_query_real": np.asarray(inputs["w_query_real"], dtype=np.float32),
                "w_query_imag": np.asarray(inputs["w_query_imag"], dtype=np.float32),
                "w_key": np.asarray(inputs["w_key"], dtype=np.float32),
                "w_value": np.asarray(inputs["w_value"], dtype=np.float32),
            }
        )
    kwargs = {}
    if _trace:
        kwargs = dict(trace=True, tmpdir="/tmp/kernel_trace")
    res = run_bass_kernel_spmd(nc, in_maps, core_ids=list(range(B)), **kwargs)
    last_exec_time_ns = res.exec_time_ns
    last_results = res
    return np.stack([res.results[b]["y"] for b in range(B)], axis=0)
